# revision 14
# baseline (speedup 1.0000x reference)
"""CapsuleLayer kernel for 8 trn2 NeuronCores.

Math (from the reference):
    c        = softmax(bias[0,:,:,0,0], axis=1)            # [I, J]
    s[b,j,d] = sum_{i,p} x[b,i,p] * W[i,j,p,d] * c[i,j]    # [B, J, D]
    out      = squash(s, axis=-1)

Folding c into W gives one big matmul
    s = X @ Wc,  X: [B, K], Wc: [K, N],  K = I*P = 32768, N = J*D = 1024.

Sharding: split the contraction dim K across the 8 cores (each core reads a
distinct 1/8 slice of W, so W's 134 MB is read exactly once fleet-wide —
the memory roofline optimum). Each core computes a partial [B, N] sum; the
host adds the 8 partials (2 MB total) and applies the tiny squash.

Precision/speed: fp32 operands are split on the host into bf16 hi/lo pairs
(x = xh + xl, Wc = wh + wl). bf16 products are exact in the PE's fp32 PSUM
accumulation, so  s = xh@wh + xh@wl + xl@wh  reproduces the fp32 matmul up
to the dropped xl@wl term (~2^-18 relative) — 3 full-rate bf16 passes
instead of the tensor engine's 4x-slower native fp32 mode. This moves the
kernel from PE-bound (~55 us) to the DMA roofline (~50 us per core).

Layout: one input tensor per core, K-tile-major. Each 128-row K-tile packs
[xh(64) | xl(64) | wh(1024) | wl(1024)] bf16 columns, so a single chunked
DMA stream feeds everything (HWDGE FIFO completes chunks in order at full
HBM bandwidth). Dummy matmuls on a memset tile pre-warm the PE's HAM clock
gate during the first chunk's DMA flight.
"""

import ml_dtypes
import numpy as np

import concourse.bass as bass
import concourse.mybir as mybir
import concourse.tile as tile
from concourse import bacc
from concourse.bass_utils import run_bass_kernel_spmd

# Problem shapes (hardcoded per contract).
B, I, P, J, D = 64, 2048, 16, 32, 32
K = I * P            # 32768 contraction
N = J * D            # 1024 output features
N_CORES = 8
K_CORE = K // N_CORES  # 4096 contraction rows per core
KT = 128               # K-tile (partition dim of one matmul)
NKT = K_CORE // KT     # 32 K-tiles per core
# Tapered DMA chunk sizes (in K-tiles). HWDGE chunks complete in issue
# order at full HBM bandwidth, so small head chunks start the PE early
# (and keep the HAM clock gate warm) while small tail chunks minimize the
# final arrival->finish latency. Total must equal NKT.
CHUNKS = [2, 2, 4, 4, 4, 4, 4, 4, 2, 2]
TC = 2 * B + 2 * N     # 2176 bf16 columns per K-tile: xh|xl|wh|wl
NB = N // 512          # PSUM-bank-sized slices of N (bank = 512 fp32)
N_WARM = 4             # dummy matmuls to lift the PE HAM clock gate

BF16 = ml_dtypes.bfloat16

_NC_CACHE = None


def _build_nc():
    """Per-core program: out[B,N] = sum over 32 K-tiles of the 3-term
    bf16-split matmul, PSUM-accumulated."""
    nc = bacc.Bacc(trn_type="TRN2", target_bir_lowering=False, debug=False)
    f32 = mybir.dt.float32
    bf16 = mybir.dt.bfloat16

    wx = nc.dram_tensor("wx", [KT, NKT * TC], bf16, kind="ExternalInput")
    out = nc.dram_tensor("out", [B, N], f32, kind="ExternalOutput")

    assert sum(CHUNKS) == NKT
    n_small = sum(1 for s in CHUNKS if s <= 2)
    n_big = sum(1 for s in CHUNKS if s > 2)
    with tile.TileContext(nc) as tc:
        with (
            tc.tile_pool(name="cpool", bufs=1) as cpool,
            # One buffer per chunk (no slot reuse) so every chunk DMA can be
            # in flight at once; split small/big pools so slots aren't all
            # padded to the largest chunk (SBUF budget).
            tc.tile_pool(name="wsmall", bufs=max(n_small, 1)) as wsmall,
            tc.tile_pool(name="wbig", bufs=max(n_big, 1)) as wbig,
            tc.tile_pool(name="opool", bufs=1) as opool,
            tc.tile_pool(name="pspool", bufs=1, space="PSUM") as pspool,
        ):
            # HAM warm-up: PE must stay busy ~3.4us to reach 2.4 GHz. These
            # dummies depend only on a memset tile, so they run during the
            # first chunk's DMA flight.
            warm = cpool.tile([KT, 512], bf16)
            nc.vector.memset(warm[:], 1.0)
            warm_ps = pspool.tile([B, 512], f32)
            for _ in range(N_WARM):
                nc.tensor.matmul(
                    warm_ps[:], warm[:, 0:B], warm[:], start=True, stop=True
                )

            ps = pspool.tile([B, N], f32)
            t = 0
            col = 0
            for csz in CHUNKS:
                pool = wsmall if csz <= 2 else wbig
                w_sb = pool.tile([KT, csz * TC], bf16)
                nc.sync.dma_start(w_sb[:], wx.ap()[:, col : col + csz * TC])
                col += csz * TC
                for tl in range(csz):
                    base = tl * TC
                    xh = w_sb[:, base : base + B]
                    xl = w_sb[:, base + B : base + 2 * B]
                    wh = base + 2 * B
                    wl = base + 2 * B + N
                    # 3-term bf16 split; lhsT grouped to pair weight loads.
                    for nb in range(NB):
                        sl = slice(nb * 512, (nb + 1) * 512)
                        nc.tensor.matmul(
                            ps[:, sl], xh, w_sb[:, wh + nb * 512 : wh + (nb + 1) * 512],
                            start=(t == 0), stop=False,
                        )
                    for nb in range(NB):
                        sl = slice(nb * 512, (nb + 1) * 512)
                        nc.tensor.matmul(
                            ps[:, sl], xh, w_sb[:, wl + nb * 512 : wl + (nb + 1) * 512],
                            start=False, stop=False,
                        )
                    for nb in range(NB):
                        sl = slice(nb * 512, (nb + 1) * 512)
                        nc.tensor.matmul(
                            ps[:, sl], xl, w_sb[:, wh + nb * 512 : wh + (nb + 1) * 512],
                            start=False, stop=(t == NKT - 1),
                        )
                    t += 1

            o_sb = opool.tile([B, N], f32)
            nc.vector.tensor_copy(o_sb[:], ps[:])
            nc.sync.dma_start(out.ap(), o_sb[:])
    # Run Bacc's compile pipeline (wait legalization, register allocation).
    # run_bass_via_pjrt serializes nc.m as-is and never finalizes.
    nc.finalize()
    return nc


def _get_nc():
    global _NC_CACHE
    if _NC_CACHE is None:
        _NC_CACHE = _build_nc()
    return _NC_CACHE


def _prepare_in_maps(inputs: np.ndarray, W: np.ndarray, bias: np.ndarray):
    """Fold softmax(bias) into W, bf16-split, pack K-tile-major, shard K."""
    x = np.asarray(inputs, dtype=np.float32)
    Wf = np.asarray(W, dtype=np.float32)
    b = np.asarray(bias, dtype=np.float32)[0, :, :, 0, 0]          # [I, J]

    # softmax over J per input capsule i (fp32, matches jax.nn.softmax).
    m = b.max(axis=1, keepdims=True)
    e = np.exp(b - m)
    c = e / e.sum(axis=1, keepdims=True)                            # [I, J]

    # Wc[(i,p),(j,d)] = W[i,j,p,d] * c[i,j]  ->  [K, N]
    wc = (Wf.transpose(0, 2, 1, 3) * c[:, None, :, None]).reshape(K, N)
    xT = np.ascontiguousarray(x.reshape(B, K).T)                    # [K, B]

    # bf16 hi/lo splits (residuals computed in fp32).
    xh = xT.astype(BF16)
    xl = (xT - xh.astype(np.float32)).astype(BF16)
    wh = wc.astype(BF16)
    wl = (wc - wh.astype(np.float32)).astype(BF16)

    packed = np.empty((K, TC), dtype=BF16)
    packed[:, 0:B] = xh
    packed[:, B : 2 * B] = xl
    packed[:, 2 * B : 2 * B + N] = wh
    packed[:, 2 * B + N :] = wl

    in_maps = []
    for cid in range(N_CORES):
        sl = slice(cid * K_CORE, (cid + 1) * K_CORE)
        # K-tile-major packing: [NKT, KT, TC] -> [KT, NKT*TC]
        core = np.ascontiguousarray(
            packed[sl].reshape(NKT, KT, TC).swapaxes(0, 1).reshape(KT, NKT * TC)
        )
        in_maps.append({"wx": core})
    return in_maps


def _squash(s: np.ndarray) -> np.ndarray:
    s2 = np.sum(np.square(s), axis=-1, keepdims=True, dtype=np.float32)
    scale = s2 / (1.0 + s2) / np.sqrt(s2)
    return (scale * s).astype(np.float32)


def run(inputs, W, bias, **spmd_kwargs):
    """Full pipeline; returns (output, BassKernelResults)."""
    in_maps = _prepare_in_maps(inputs, W, bias)
    res = run_bass_kernel_spmd(
        _get_nc(), in_maps, core_ids=list(range(N_CORES)), **spmd_kwargs
    )
    s = np.zeros((B, N), dtype=np.float32)
    for r in res.results:
        s += np.asarray(r["out"], dtype=np.float32)
    out = _squash(s.reshape(B, J, D))
    return out, res


def kernel(inputs, W, bias):
    out, _ = run(inputs, W, bias)
    return out
